# revision 1
# baseline (speedup 1.0000x reference)
"""LoRA MLP (gate_up + SiLU*up + down, each with rank-16 LoRA) on 8 TRN2 cores.

Strategy: pure data-parallel over tokens (16384 = 8 x 2048); weights are
replicated to every core, so no collectives are needed. The rank-16 LoRA is
merged into the base weights host-side (W_eff = W + A @ B, the standard
merged-adapter serving trick), so the device kernel is a plain dense MLP.
All matmul operands are bf16: full PE rate, and bf16 stationaries get fast
weight load so LDWEIGHTS hides completely under the 512-col matmuls (fp32r
weights cannot use FWL and leave ~180ns of exposed weight-load per matmul).
Activations stay transposed ([feature, token]) so every matmul consumes
natural-layout weights; accumulation is fp32 in PSUM.
"""

import numpy as np
import ml_dtypes

import concourse.mybir as mybir
import concourse.tile as tile
from concourse import bacc
from concourse.bass_utils import run_bass_kernel_spmd

TOKENS, D, FF, R = 16384, 1024, 2816, 16
N_CORES = 8
T_CORE = TOKENS // N_CORES  # 2048
TSUB = 512                  # psum free-dim tile (1 bank fp32)
DT = D // 128               # 8 d-model tiles
FFT = FF // 128             # 22 ff tiles
F32 = mybir.dt.float32
BF16 = mybir.dt.bfloat16
SILU = mybir.ActivationFunctionType.Silu
COPY = mybir.ActivationFunctionType.Copy

_prog_cache = {}


def _build():
    nc = bacc.Bacc("TRN2", target_bir_lowering=False, debug=False)
    xT = nc.dram_tensor("xT", [D, T_CORE], BF16, kind="ExternalInput").ap()
    w1 = nc.dram_tensor("W1", [D, 2 * FF], BF16, kind="ExternalInput").ap()
    w2 = nc.dram_tensor("W2", [FF, D], BF16, kind="ExternalInput").ap()
    out = nc.dram_tensor("out", [T_CORE, D], BF16, kind="ExternalOutput").ap()

    w1r = w1.rearrange("(dt p) f -> p dt f", p=128)   # [128, 8, 5632]
    w2r = w2.rearrange("(ft p) d -> p ft d", p=128)   # [128, 22, 1024]
    xTr = xT.rearrange("(dt p) t -> p dt t", p=128)   # [128, 8, 2048]

    with tile.TileContext(nc) as tc:
        with (
            tc.tile_pool(name="xp", bufs=1) as xp,
            tc.tile_pool(name="hp", bufs=1) as hp,
            tc.tile_pool(name="w1p", bufs=2) as w1p,
            tc.tile_pool(name="w2p", bufs=2) as w2p,
            tc.tile_pool(name="evp", bufs=2) as evp,
            tc.tile_pool(name="ps", bufs=1, space="PSUM") as ps,
        ):
            NPRE = 7  # i-tiles swept ts-outer while x streams in
            w1_tiles = {}

            def w1_dma(i):
                g = w1p.tile([128, DT, 128], BF16, tag="w1g", bufs=NPRE + 1)
                nc.sync.dma_start(g[:], w1r[:, :, i * 128 : (i + 1) * 128])
                u = w1p.tile([128, DT, 128], BF16, tag="w1u", bufs=NPRE + 1)
                nc.sync.dma_start(u[:], w1r[:, :, FF + i * 128 : FF + (i + 1) * 128])
                w1_tiles[i] = (g, u)

            # DMA issue order: first weight tile, then x chunk 0 (unblocks the
            # first chain ASAP), then the remaining pre-set weights interleaved
            # ahead of the later x chunks.
            xt_sb = xp.tile([128, DT, T_CORE], BF16, tag="xt")

            def xt_dma(ts, eng=None):
                eng = eng or nc.sync
                tsl = slice(ts * TSUB, (ts + 1) * TSUB)
                for d in range(DT):
                    eng.dma_start(xt_sb[:, d, tsl], xTr[:, d, tsl])

            # Tiny first DMA (128KB) so the PE warmup can begin ~1us after
            # DMA flow starts, ahead of the bulkier w1/x transfers.
            wu = w1p.tile([128, TSUB], BF16, tag="warm", bufs=1)
            nc.sync.dma_start(wu[:], w1r[:, 0, 0:TSUB])

            w1_dma(0)
            xt_dma(0)
            for i in range(1, NPRE):
                w1_dma(i)
            for ts in range(1, T_CORE // TSUB):
                xt_dma(ts)

            # Warm the PE clock gate (HAM) during the x DMA: scratch matmuls
            # into a psum bank reused by po.
            pw = ps.tile([128, TSUB], F32, tag="po", bufs=2)
            for _ in range(8):
                nc.tensor.matmul(
                    pw[:], wu[:, 0:128], wu[:],
                    start=True, stop=True,
                )

            # ---- phase 1: h^T = silu(gate^T) * up^T ----
            h_sb = hp.tile([128, FFT, T_CORE], BF16, tag="h")

            def chains(i, ts):
                w1g, w1u = w1_tiles[i]
                tsl = slice(ts * TSUB, (ts + 1) * TSUB)
                pg = ps.tile([128, TSUB], F32, tag="pg", bufs=3)
                for d in range(DT):
                    nc.tensor.matmul(
                        pg[:], w1g[:, d, :], xt_sb[:, d, tsl],
                        start=(d == 0), stop=(d == DT - 1),
                    )
                pu = ps.tile([128, TSUB], F32, tag="pu", bufs=3)
                for d in range(DT):
                    nc.tensor.matmul(
                        pu[:], w1u[:, d, :], xt_sb[:, d, tsl],
                        start=(d == 0), stop=(d == DT - 1),
                    )
                tmp = evp.tile([128, TSUB], F32, tag="tmp")
                nc.scalar.activation(tmp[:], pg[:], SILU)
                nc.vector.tensor_mul(h_sb[:, i, tsl], tmp[:], pu[:])

            # ramp: sweep the pre-loaded i-tiles ts-outer so compute on x
            # chunk 0 hides the arrival of chunks 1..3
            for ts in range(T_CORE // TSUB):
                for i in range(NPRE):
                    chains(i, ts)
            # steady state: i-outer with one-ahead weight prefetch
            for i in range(NPRE, FFT):
                if i not in w1_tiles:
                    w1_dma(i)
                if i + 1 < FFT and i + 1 not in w1_tiles:
                    w1_dma(i + 1)
                for ts in range(T_CORE // TSUB):
                    chains(i, ts)

            # ---- phase 2: out = h^T.T @ W2 ----
            for dh in range(D // TSUB):
                dsl = slice(dh * TSUB, (dh + 1) * TSUB)
                w2_sb = w2p.tile([128, FFT, TSUB], BF16, tag="w2")
                for i in range(FFT):
                    nc.sync.dma_start(w2_sb[:, i, :], w2r[:, i, dsl])
                for tt in range(T_CORE // 128):
                    ttl = slice(tt * 128, (tt + 1) * 128)
                    last = dh == D // TSUB - 1 and tt == T_CORE // 128 - 1
                    if not last:
                        po = ps.tile([128, TSUB], F32, tag="po", bufs=2)
                        for i in range(FFT):
                            nc.tensor.matmul(
                                po[:], h_sb[:, i, ttl], w2_sb[:, i, :],
                                start=(i == 0), stop=(i == FFT - 1),
                            )
                        o_sb = evp.tile([128, TSUB], BF16, tag="o")
                        nc.scalar.activation(o_sb[:], po[:], COPY)
                        nc.sync.dma_start(out[ttl, dsl], o_sb[:])
                    else:
                        # split the final tile in two so its ACT+store
                        # overlaps the second half's matmul chain
                        for hoff in (0, TSUB // 2):
                            po = ps.tile([128, TSUB], F32, tag="po", bufs=2)
                            for i in range(FFT):
                                nc.tensor.matmul(
                                    po[:, 0 : TSUB // 2],
                                    h_sb[:, i, ttl],
                                    w2_sb[:, i, hoff : hoff + TSUB // 2],
                                    start=(i == 0), stop=(i == FFT - 1),
                                )
                            oh = evp.tile([128, TSUB // 2], BF16, tag="o2")
                            nc.scalar.activation(oh[:], po[:, 0 : TSUB // 2], COPY)
                            nc.sync.dma_start(
                                out[ttl, dh * TSUB + hoff : dh * TSUB + hoff + TSUB // 2],
                                oh[:],
                            )
    nc.compile()
    return nc


def _get_prog():
    if "nc" not in _prog_cache:
        _prog_cache["nc"] = _build()
    return _prog_cache["nc"]


def run_sharded(inputs, trace=False):
    nc = _get_prog()
    bf16 = ml_dtypes.bfloat16
    x = np.asarray(inputs["x"], dtype=np.float32)
    # merge the rank-16 LoRA into the base weights (W_eff = W + A @ B)
    w1 = (
        np.asarray(inputs["W_gate_up"], dtype=np.float32)
        + np.asarray(inputs["A_gate_up"], dtype=np.float32)
        @ np.asarray(inputs["B_gate_up"], dtype=np.float32)
    ).astype(bf16)
    w2 = (
        np.asarray(inputs["W_down"], dtype=np.float32)
        + np.asarray(inputs["A_down"], dtype=np.float32)
        @ np.asarray(inputs["B_down"], dtype=np.float32)
    ).astype(bf16)
    weights = {"W1": np.ascontiguousarray(w1), "W2": np.ascontiguousarray(w2)}
    in_maps = []
    for c in range(N_CORES):
        xs = np.ascontiguousarray(x[c * T_CORE : (c + 1) * T_CORE].T.astype(bf16))
        in_maps.append({"xT": xs, **weights})
    res = run_bass_kernel_spmd(nc, in_maps, list(range(N_CORES)), trace=trace)
    outs = [np.asarray(res.results[c]["out"], dtype=np.float32) for c in range(N_CORES)]
    full = np.concatenate(outs, axis=0)
    return full, res


def kernel(**inputs):
    full, _ = run_sharded(inputs, trace=False)
    return full



# revision 5
# speedup vs baseline: 1.0007x; 1.0007x over previous
"""LoRA MLP (gate_up + SiLU*up + down, each with rank-16 LoRA) on 8 TRN2 cores.

Strategy: pure data-parallel over tokens (16384 = 8 x 2048); weights are
replicated to every core, so no collectives are needed. The rank-16 LoRA is
merged into the base weights host-side (W_eff = W + A @ B, the standard
merged-adapter serving trick), so the device kernel is a plain dense MLP.
All matmul operands are bf16: full PE rate, and bf16 stationaries get fast
weight load so LDWEIGHTS hides under the 512-col matmuls. Activations stay
transposed ([feature, token]) so every matmul consumes natural-layout
weights; accumulation is fp32 in PSUM.

W1 is repacked host-side to [128, 22, 8, 256] (partition, ff-tile, d-tile,
gate|up) so each ff-tile's weights are one contiguous-per-partition 4KB DMA:
inner runs >= 512B avoid the DMA read-modify-write penalty, and one DMA per
tile (instead of two strided ones) halves descriptor+instruction overhead.
x chunks and w2 use single 3D DMAs for the same reason.
"""

import numpy as np
import ml_dtypes

import concourse.mybir as mybir
import concourse.tile as tile
from concourse import bacc
from concourse.bass_utils import run_bass_kernel_spmd

TOKENS, D, FF, R = 16384, 1024, 2816, 16
N_CORES = 8
T_CORE = TOKENS // N_CORES  # 2048
TSUB = 512                  # psum free-dim tile (1 bank fp32)
DT = D // 128               # 8 d-model tiles
FFT = FF // 128             # 22 ff tiles
F32 = mybir.dt.float32
BF16 = mybir.dt.bfloat16
SILU = mybir.ActivationFunctionType.Silu
COPY = mybir.ActivationFunctionType.Copy

_prog_cache = {}


def _build(iters=1):
    nc = bacc.Bacc("TRN2", target_bir_lowering=False, debug=False)
    xT = nc.dram_tensor("xT", [D, T_CORE], BF16, kind="ExternalInput").ap()
    w1 = nc.dram_tensor("W1", [128, FFT, DT, 256], BF16, kind="ExternalInput").ap()
    w2 = nc.dram_tensor("W2", [FF, D], BF16, kind="ExternalInput").ap()
    out = nc.dram_tensor("out", [T_CORE, D], BF16, kind="ExternalOutput").ap()

    w2r = w2.rearrange("(ft p) d -> p ft d", p=128)   # [128, 22, 1024]
    xTr = xT.rearrange("(dt p) t -> p dt t", p=128)   # [128, 8, 2048]

    with tile.TileContext(nc) as tc:
        with (
            tc.tile_pool(name="xp", bufs=1) as xp,
            tc.tile_pool(name="hp", bufs=1) as hp,
            tc.tile_pool(name="w1p", bufs=2) as w1p,
            tc.tile_pool(name="w2p", bufs=2) as w2p,
            tc.tile_pool(name="evp", bufs=2) as evp,
            tc.tile_pool(name="ps", bufs=1, space="PSUM") as ps,
        ):
          for _it in range(iters):
            NPRE = 7  # i-tiles swept ts-outer while x streams in
            w1_tiles = {}

            def w1_dma(i):
                t = w1p.tile([128, DT, 256], BF16, tag="w1", bufs=NPRE + 1)
                nc.sync.dma_start(t[:], w1[:, i, :, :])
                w1_tiles[i] = t

            xt_sb = xp.tile([128, DT, T_CORE], BF16, tag="xt")

            def xt_dma(ts, split=False):
                tsl = slice(ts * TSUB, (ts + 1) * TSUB)
                if split:
                    for d in range(DT):
                        nc.sync.dma_start(xt_sb[:, d, tsl], xTr[:, d, tsl])
                else:
                    nc.sync.dma_start(xt_sb[:, :, tsl], xTr[:, :, tsl])

            if _it == 0:
                # Tiny first DMA (64KB) so the PE warmup can begin well under
                # 1us after DMA flow starts, ahead of the bulk w1/x traffic.
                wu = w1p.tile([128, 256], BF16, tag="warm", bufs=1)
                nc.sync.dma_start(wu[:], w1[:, 0, 0, :])

            # First w1 tile, then x chunk 0 streamed per d-tile (so the first
            # chain can start consuming while later d-tiles are in flight),
            # then the remaining pre-set weights interleaved with x chunks.
            w1_dma(0)
            xt_dma(0, split=True)
            w1_dma(1)
            xt_dma(1)
            for i in range(2, 4):
                w1_dma(i)
            xt_dma(2)
            for i in range(4, 6):
                w1_dma(i)
            xt_dma(3)
            w1_dma(6)

            if _it == 0:
                # Warm the PE clock gate (HAM) during the x DMA: scratch
                # matmuls into a psum bank reused by po.
                pw = ps.tile([128, 256], F32, tag="po", bufs=2)
                for _ in range(8):
                    nc.tensor.matmul(
                        pw[:], wu[:, 0:128], wu[:],
                        start=True, stop=True,
                    )

            # ---- phase 1: h^T = silu(gate^T) * up^T ----
            h_sb = hp.tile([128, FFT, T_CORE], BF16, tag="h")

            def chains(i, ts):
                w1t = w1_tiles[i]
                tsl = slice(ts * TSUB, (ts + 1) * TSUB)
                pg = ps.tile([128, TSUB], F32, tag="pg", bufs=3)
                for d in range(DT):
                    nc.tensor.matmul(
                        pg[:], w1t[:, d, 0:128], xt_sb[:, d, tsl],
                        start=(d == 0), stop=(d == DT - 1),
                    )
                pu = ps.tile([128, TSUB], F32, tag="pu", bufs=3)
                for d in range(DT):
                    nc.tensor.matmul(
                        pu[:], w1t[:, d, 128:256], xt_sb[:, d, tsl],
                        start=(d == 0), stop=(d == DT - 1),
                    )
                tmp = evp.tile([128, TSUB], F32, tag="tmp")
                nc.scalar.activation(tmp[:], pg[:], SILU)
                nc.vector.tensor_mul(h_sb[:, i, tsl], tmp[:], pu[:])

            # ramp: sweep the pre-loaded i-tiles ts-outer so compute on x
            # chunk 0 hides the arrival of chunks 1..3
            for ts in range(T_CORE // TSUB):
                for i in range(NPRE):
                    chains(i, ts)
            # steady state: i-outer with one-ahead weight prefetch
            for i in range(NPRE, FFT):
                if i not in w1_tiles:
                    w1_dma(i)
                if i + 1 < FFT and i + 1 not in w1_tiles:
                    w1_dma(i + 1)
                for ts in range(T_CORE // TSUB):
                    chains(i, ts)

            # ---- phase 2: out = h^T.T @ W2 ----
            for dh in range(D // TSUB):
                dsl = slice(dh * TSUB, (dh + 1) * TSUB)
                w2_sb = w2p.tile([128, FFT, TSUB], BF16, tag="w2")
                nc.sync.dma_start(w2_sb[:], w2r[:, :, dsl])
                for tt in range(T_CORE // 128):
                    ttl = slice(tt * 128, (tt + 1) * 128)
                    last = dh == D // TSUB - 1 and tt == T_CORE // 128 - 1
                    if not last:
                        po = ps.tile([128, TSUB], F32, tag="po", bufs=2)
                        for i in range(FFT):
                            nc.tensor.matmul(
                                po[:], h_sb[:, i, ttl], w2_sb[:, i, :],
                                start=(i == 0), stop=(i == FFT - 1),
                            )
                        o_sb = evp.tile([128, TSUB], BF16, tag="o")
                        nc.scalar.activation(o_sb[:], po[:], COPY)
                        nc.sync.dma_start(out[ttl, dsl], o_sb[:])
                    else:
                        # taper the final tile so its ACT+store overlaps the
                        # remaining matmul chains
                        for hoff, hw_ in ((0, 256), (256, 128), (384, 128)):
                            po = ps.tile([128, TSUB], F32, tag="po", bufs=2)
                            for i in range(FFT):
                                nc.tensor.matmul(
                                    po[:, 0:hw_],
                                    h_sb[:, i, ttl],
                                    w2_sb[:, i, hoff : hoff + hw_],
                                    start=(i == 0), stop=(i == FFT - 1),
                                )
                            oh = evp.tile([128, hw_], BF16, tag="o2")
                            nc.scalar.activation(oh[:], po[:, 0:hw_], COPY)
                            nc.sync.dma_start(
                                out[ttl, dh * TSUB + hoff : dh * TSUB + hoff + hw_],
                                oh[:],
                            )
    nc.compile()
    return nc


def _get_prog():
    if "nc" not in _prog_cache:
        _prog_cache["nc"] = _build()
    return _prog_cache["nc"]


def _pack_w1(w1_eff):
    """[1024, 5632] -> [128, 22, 8, 256] with gate|up interleaved per d-tile."""
    g = w1_eff[:, :FF].reshape(DT, 128, FFT, 128)
    u = w1_eff[:, FF:].reshape(DT, 128, FFT, 128)
    pk = np.concatenate([g, u], axis=-1)          # [8, 128, 22, 256]
    return np.ascontiguousarray(pk.transpose(1, 2, 0, 3))  # [128, 22, 8, 256]


def run_sharded(inputs, trace=False):
    nc = _get_prog()
    bf16 = ml_dtypes.bfloat16
    x = np.asarray(inputs["x"], dtype=np.float32)
    # merge the rank-16 LoRA into the base weights (W_eff = W + A @ B)
    w1 = (
        np.asarray(inputs["W_gate_up"], dtype=np.float32)
        + np.asarray(inputs["A_gate_up"], dtype=np.float32)
        @ np.asarray(inputs["B_gate_up"], dtype=np.float32)
    ).astype(bf16)
    w2 = (
        np.asarray(inputs["W_down"], dtype=np.float32)
        + np.asarray(inputs["A_down"], dtype=np.float32)
        @ np.asarray(inputs["B_down"], dtype=np.float32)
    ).astype(bf16)
    weights = {"W1": _pack_w1(w1), "W2": np.ascontiguousarray(w2)}
    in_maps = []
    for c in range(N_CORES):
        xs = np.ascontiguousarray(x[c * T_CORE : (c + 1) * T_CORE].T.astype(bf16))
        in_maps.append({"xT": xs, **weights})
    res = run_bass_kernel_spmd(nc, in_maps, list(range(N_CORES)), trace=trace)
    outs = [np.asarray(res.results[c]["out"], dtype=np.float32) for c in range(N_CORES)]
    full = np.concatenate(outs, axis=0)
    return full, res


def kernel(**inputs):
    full, _ = run_sharded(inputs, trace=False)
    return full


# revision 8
# speedup vs baseline: 1.0062x; 1.0055x over previous
"""LoRA MLP (gate_up + SiLU*up + down, each with rank-16 LoRA) on 8 TRN2 cores.

Strategy: pure data-parallel over tokens (16384 = 8 x 2048); weights are
replicated to every core, so no collectives are needed. The rank-16 LoRA is
merged into the base weights host-side (W_eff = W + A @ B, the standard
merged-adapter serving trick), so the device kernel is a plain dense MLP.
All matmul operands are bf16: full PE rate, and bf16 stationaries get fast
weight load so LDWEIGHTS hides under the 512-col matmuls. Activations stay
transposed ([feature, token]) so every matmul consumes natural-layout
weights; accumulation is fp32 in PSUM.

W1 is repacked host-side to [128, 22, 8, 256] (partition, ff-tile, d-tile,
gate|up) so each ff-tile's weights are one contiguous-per-partition 4KB DMA:
inner runs >= 512B avoid the DMA read-modify-write penalty, and one DMA per
tile (instead of two strided ones) halves descriptor+instruction overhead.
x chunks and w2 use single 3D DMAs for the same reason.
"""

import numpy as np
import ml_dtypes

import concourse.mybir as mybir
import concourse.tile as tile
from concourse import bacc
from concourse.bass_utils import run_bass_kernel_spmd

TOKENS, D, FF, R = 16384, 1024, 2816, 16
N_CORES = 8
T_CORE = TOKENS // N_CORES  # 2048
TSUB = 512                  # psum free-dim tile (1 bank fp32)
DT = D // 128               # 8 d-model tiles
FFT = FF // 128             # 22 ff tiles
F32 = mybir.dt.float32
BF16 = mybir.dt.bfloat16
SILU = mybir.ActivationFunctionType.Silu
COPY = mybir.ActivationFunctionType.Copy

_prog_cache = {}


def _build(iters=1):
    nc = bacc.Bacc("TRN2", target_bir_lowering=False, debug=False)
    xT = nc.dram_tensor("xT", [D, T_CORE], BF16, kind="ExternalInput").ap()
    w1 = nc.dram_tensor("W1", [128, FFT, DT, 256], BF16, kind="ExternalInput").ap()
    w2 = nc.dram_tensor("W2", [FF, D], BF16, kind="ExternalInput").ap()
    out = nc.dram_tensor("out", [T_CORE, D], BF16, kind="ExternalOutput").ap()

    w2r = w2.rearrange("(ft p) d -> p ft d", p=128)   # [128, 22, 1024]
    xTr = xT.rearrange("(dt p) t -> p dt t", p=128)   # [128, 8, 2048]

    with tile.TileContext(nc) as tc:
        with (
            tc.tile_pool(name="xp", bufs=1) as xp,
            tc.tile_pool(name="hp", bufs=1) as hp,
            tc.tile_pool(name="w1p", bufs=2) as w1p,
            tc.tile_pool(name="w2p", bufs=2) as w2p,
            tc.tile_pool(name="evp", bufs=2) as evp,
            tc.tile_pool(name="ps", bufs=1, space="PSUM") as ps,
        ):
          for _it in range(iters):
            NPRE = 7  # i-tiles swept ts-outer while x streams in
            w1_tiles = {}

            def w1_dma(i):
                t = w1p.tile([128, DT, 256], BF16, tag="w1", bufs=NPRE + 1)
                nc.sync.dma_start(t[:], w1[:, i, :, :])
                w1_tiles[i] = t

            xt_sb = xp.tile([128, DT, T_CORE], BF16, tag="xt")

            def xt_dma(ts, half=None):
                tsl = slice(ts * TSUB, (ts + 1) * TSUB)
                if half is None:
                    nc.sync.dma_start(xt_sb[:, :, tsl], xTr[:, :, tsl])
                else:
                    dsl = slice(half * DT // 2, (half + 1) * DT // 2)
                    nc.sync.dma_start(xt_sb[:, dsl, tsl], xTr[:, dsl, tsl])

            if _it == 0:
                # Memset (no DMA dependency) so the PE warmup can begin
                # immediately, well before the first weight tile lands.
                wu = w1p.tile([128, 512], BF16, tag="warm", bufs=1)
                nc.vector.memset(wu[:], 0)

            # DMA issue order tuned against chain consumption: chain(i,ts=0)
            # costs ~3.4us of PE; w1 tiles arrive every ~1.6us, x chunks
            # every ~3.2us, all on one serial queue. w1 tile i must land by
            # ~5.8 + 3.4*i us; x chunk ts by ~5.8 + (7*ts+...)*3.4.
            w1_dma(0)
            xt_dma(0, half=0)
            xt_dma(0, half=1)
            for i in range(1, NPRE):
                w1_dma(i)
            for ts in range(1, T_CORE // TSUB):
                xt_dma(ts)

            if _it == 0:
                # Warm the PE clock gate (HAM) during the x DMA: scratch
                # matmuls into a psum bank reused by po.
                pw = ps.tile([128, TSUB], F32, tag="po", bufs=2)
                for _ in range(8):
                    nc.tensor.matmul(
                        pw[:], wu[:, 0:128], wu[:],
                        start=True, stop=True,
                    )

            # ---- phase 1: h^T = silu(gate^T) * up^T ----
            h_sb = hp.tile([128, FFT, T_CORE], BF16, tag="h")

            def chains(i, ts):
                w1t = w1_tiles[i]
                tsl = slice(ts * TSUB, (ts + 1) * TSUB)
                pg = ps.tile([128, TSUB], F32, tag="pg", bufs=3)
                for d in range(DT):
                    nc.tensor.matmul(
                        pg[:], w1t[:, d, 0:128], xt_sb[:, d, tsl],
                        start=(d == 0), stop=(d == DT - 1),
                    )
                pu = ps.tile([128, TSUB], F32, tag="pu", bufs=3)
                for d in range(DT):
                    nc.tensor.matmul(
                        pu[:], w1t[:, d, 128:256], xt_sb[:, d, tsl],
                        start=(d == 0), stop=(d == DT - 1),
                    )
                tmp = evp.tile([128, TSUB], F32, tag="tmp")
                nc.scalar.activation(tmp[:], pg[:], SILU)
                nc.vector.tensor_mul(h_sb[:, i, tsl], tmp[:], pu[:])

            # ramp: sweep the pre-loaded i-tiles ts-outer so compute on x
            # chunk 0 hides the arrival of chunks 1..3
            for ts in range(T_CORE // TSUB):
                for i in range(NPRE):
                    chains(i, ts)
            # steady state: i-outer with one-ahead weight prefetch
            for i in range(NPRE, FFT):
                if i not in w1_tiles:
                    w1_dma(i)
                if i + 1 < FFT and i + 1 not in w1_tiles:
                    w1_dma(i + 1)
                for ts in range(T_CORE // TSUB):
                    chains(i, ts)

            # ---- phase 2: out = h^T.T @ W2 ----
            for dh in range(D // TSUB):
                dsl = slice(dh * TSUB, (dh + 1) * TSUB)
                w2_sb = w2p.tile([128, FFT, TSUB], BF16, tag="w2")
                nc.sync.dma_start(w2_sb[:], w2r[:, :, dsl])
                for tt in range(T_CORE // 128):
                    ttl = slice(tt * 128, (tt + 1) * 128)
                    last = dh == D // TSUB - 1 and tt == T_CORE // 128 - 1
                    if not last:
                        po = ps.tile([128, TSUB], F32, tag="po", bufs=2)
                        for i in range(FFT):
                            nc.tensor.matmul(
                                po[:], h_sb[:, i, ttl], w2_sb[:, i, :],
                                start=(i == 0), stop=(i == FFT - 1),
                            )
                        o_sb = evp.tile([128, TSUB], BF16, tag="o")
                        nc.scalar.activation(o_sb[:], po[:], COPY)
                        nc.sync.dma_start(out[ttl, dsl], o_sb[:])
                    else:
                        # taper the final tile so its ACT+store overlaps the
                        # remaining matmul chains
                        for hoff, hw_ in ((0, 256), (256, 128), (384, 128)):
                            po = ps.tile([128, TSUB], F32, tag="po", bufs=2)
                            for i in range(FFT):
                                nc.tensor.matmul(
                                    po[:, 0:hw_],
                                    h_sb[:, i, ttl],
                                    w2_sb[:, i, hoff : hoff + hw_],
                                    start=(i == 0), stop=(i == FFT - 1),
                                )
                            oh = evp.tile([128, hw_], BF16, tag="o2")
                            nc.scalar.activation(oh[:], po[:, 0:hw_], COPY)
                            nc.sync.dma_start(
                                out[ttl, dh * TSUB + hoff : dh * TSUB + hoff + hw_],
                                oh[:],
                            )
    nc.compile()
    return nc


def _get_prog():
    if "nc" not in _prog_cache:
        _prog_cache["nc"] = _build()
    return _prog_cache["nc"]


def _pack_w1(w1_eff):
    """[1024, 5632] -> [128, 22, 8, 256] with gate|up interleaved per d-tile."""
    g = w1_eff[:, :FF].reshape(DT, 128, FFT, 128)
    u = w1_eff[:, FF:].reshape(DT, 128, FFT, 128)
    pk = np.concatenate([g, u], axis=-1)          # [8, 128, 22, 256]
    return np.ascontiguousarray(pk.transpose(1, 2, 0, 3))  # [128, 22, 8, 256]


def run_sharded(inputs, trace=False):
    nc = _get_prog()
    bf16 = ml_dtypes.bfloat16
    x = np.asarray(inputs["x"], dtype=np.float32)
    # merge the rank-16 LoRA into the base weights (W_eff = W + A @ B)
    w1 = (
        np.asarray(inputs["W_gate_up"], dtype=np.float32)
        + np.asarray(inputs["A_gate_up"], dtype=np.float32)
        @ np.asarray(inputs["B_gate_up"], dtype=np.float32)
    ).astype(bf16)
    w2 = (
        np.asarray(inputs["W_down"], dtype=np.float32)
        + np.asarray(inputs["A_down"], dtype=np.float32)
        @ np.asarray(inputs["B_down"], dtype=np.float32)
    ).astype(bf16)
    weights = {"W1": _pack_w1(w1), "W2": np.ascontiguousarray(w2)}
    in_maps = []
    for c in range(N_CORES):
        xs = np.ascontiguousarray(x[c * T_CORE : (c + 1) * T_CORE].T.astype(bf16))
        in_maps.append({"xT": xs, **weights})
    res = run_bass_kernel_spmd(nc, in_maps, list(range(N_CORES)), trace=trace)
    outs = [np.asarray(res.results[c]["out"], dtype=np.float32) for c in range(N_CORES)]
    full = np.concatenate(outs, axis=0)
    return full, res


def kernel(**inputs):
    full, _ = run_sharded(inputs, trace=False)
    return full
